# revision 11
# baseline (speedup 1.0000x reference)
"""Trainium2 Bass kernel for nn_Damping (B=32768, N=64, H=256).

Per-sample computation:
    diag = (relu(MLP_d(x)) + damp_min) * x          # [64]
    off  = MLP_o(x)                                  # [2016] strictly-lower entries
    L    = scatter(off -> strict lower, diag -> diagonal)   # [64, 64]
    out  = L @ (L^T @ x)

Strategy: pure data parallel over 8 NeuronCores (4096 samples each).
On-chip layout is feature-major: activations live as [features(partitions),
batch(free)] tiles of 512 samples.  x arrives pre-transposed from the host
(xt, bf16) and the pass-1 row-expansion xrep[p] = x[row(p)] is pre-gathered
on the host and streamed from DRAM.  The pass-2 column-expansion
vrep[p] = v[col(p)] is produced by a DMA gather (v roundtrips through a
small DRAM buffer; gpsimd SWDGE gathers rows by a static index table), so
the PE runs no transpose or expansion matmuls at all — only the MLPs, the
Woo projection and the Ecol/Erow 0/1-reduction matmuls:
    v   = Ecol^T @ (off ⊙ xrep) + diag ⊙ x          (v = L^T x)
    out = Erow^T @ (off ⊙ vrep) + diag ⊙ v          (out = L v)
All elementwise multiplies are bf16 SBUF-only, hitting the DVE fast path.
Block b's pass 2 is software-pipelined one block behind pass 1 (slice loops
merged) so the gather latency hides under the next block's pass-1 work.
The output is stored feature-major [64, 4096] and transposed on the host.
"""

import numpy as np

B, N, H, OFF = 32768, 64, 256, 2016
NCORES = 8
BLOCAL = B // NCORES          # 4096 samples per core
NSLICES = 16
SL = 128                      # padded slice width; 16*128 = 2048
OFFP = NSLICES * SL           # 2048 (padded off dim)
NBLOCKS = 8                   # blocks of 512 samples per core
BT = 512                      # batch tile (moving free dim)
NCHUNK = 4                    # vrep gather chunks per block (4 slices each)
CSL = NSLICES // NCHUNK       # slices per gather chunk

_compiled = None


def _build_program():
    import concourse.bass as bass  # noqa: F401
    import concourse.mybir as mybir
    import concourse.tile as tile
    from concourse import bacc

    f32 = mybir.dt.float32
    bf16 = mybir.dt.bfloat16
    i16 = mybir.dt.int16
    AF = mybir.ActivationFunctionType

    nc = bacc.Bacc("TRN2", target_bir_lowering=False, debug=False,
                   num_devices=NCORES)

    def din(name, shape, dt=f32):
        return nc.dram_tensor(name, list(shape), dt, kind="ExternalInput").ap()

    xt_ap = din("xt", (N, BLOCAL), bf16)
    xrep_ap = din("xrep", (NBLOCKS, SL, NSLICES, BT), bf16)
    wd1_ap = din("wd1", (N, H), bf16)
    wd2_ap = din("wd2", (128, 2, H), bf16)
    wdo_ap = din("wdo", (128, 2, N), bf16)
    wo1_ap = din("wo1", (N, H), bf16)
    wo2_ap = din("wo2", (128, 2, H), bf16)
    woo_ap = din("woo", (128, 2, OFFP), bf16)
    bd1_ap = din("bd1", (128, 2))
    bd2_ap = din("bd2", (128, 2))
    bo1_ap = din("bo1", (128, 2))
    bo2_ap = din("bo2", (128, 2))
    bdo_ap = din("bdo", (N, 1))
    boo_ap = din("boo", (SL, NSLICES))
    dm_ap = din("dm", (N, 1))
    cidx_ap = din("cidx", (128, 128), i16)
    ecol_ap = din("ecol", (SL, NSLICES * N), bf16)
    erow_ap = din("erow", (SL, NSLICES * N), bf16)
    out_ap = nc.dram_tensor("out", [N, BLOCAL], f32, kind="ExternalOutput").ap()

    with tile.TileContext(nc) as tc:
        with (
            tc.tile_pool(name="consts", bufs=1) as consts,
            tc.tile_pool(name="xrp", bufs=2) as xr_pool,
            tc.tile_pool(name="vrp", bufs=2) as vr_pool,
            tc.tile_pool(name="vdram", bufs=2, space="DRAM") as vd_pool,
            tc.tile_pool(name="acts", bufs=2) as act_pool,
            tc.tile_pool(name="offp", bufs=2) as off_pool,
            tc.tile_pool(name="mp", bufs=2) as m_pool,
            tc.tile_pool(name="small", bufs=2) as small_pool,
            tc.tile_pool(name="ps_mlp", bufs=3, space="PSUM") as ps_mlp,
            tc.tile_pool(name="ps_off", bufs=3, space="PSUM") as ps_off,
            tc.tile_pool(name="ps_acc", bufs=2, space="PSUM") as ps_acc,
        ):
            # ---- load constants ----
            def load(name, shape, ap):
                t = consts.tile(list(shape), ap.dtype, tag=name)
                nc.sync.dma_start(t[:], ap)
                return t

            xt = load("xt", (N, BLOCAL), xt_ap)
            wd1 = load("wd1", (N, H), wd1_ap)
            wd2 = load("wd2", (128, 2, H), wd2_ap)
            wdo = load("wdo", (128, 2, N), wdo_ap)
            wo1 = load("wo1", (N, H), wo1_ap)
            wo2 = load("wo2", (128, 2, H), wo2_ap)
            woo = load("woo", (128, 2, OFFP), woo_ap)
            bd1 = load("bd1", (128, 2), bd1_ap)
            bd2 = load("bd2", (128, 2), bd2_ap)
            bo1 = load("bo1", (128, 2), bo1_ap)
            bo2 = load("bo2", (128, 2), bo2_ap)
            bdo = load("bdo", (N, 1), bdo_ap)
            boo = load("boo", (SL, NSLICES), boo_ap)
            dm = load("dm", (N, 1), dm_ap)
            cidx = load("cidx", (128, 128), cidx_ap)
            ecol = load("ecol", (SL, NSLICES * N), ecol_ap)
            erow = load("erow", (SL, NSLICES * N), erow_ap)

            def mlp2(w1, b1, w2, b2, xT, tag):
                """Two tanh layers; returns [128, 2, 512] feature-major bf16."""
                a1 = act_pool.tile([128, 2, BT], bf16, tag=tag + "1")
                for s in range(2):
                    ps = ps_mlp.tile([128, BT], f32, tag="mlp")
                    nc.tensor.matmul(ps[:], w1[:, 128 * s:128 * (s + 1)],
                                     xT, start=True, stop=True)
                    nc.scalar.activation(a1[:, s], ps[:], AF.Tanh,
                                         bias=b1[:, s:s + 1])
                a2 = act_pool.tile([128, 2, BT], bf16, tag=tag + "2")
                for s in range(2):
                    ps = ps_mlp.tile([128, BT], f32, tag="mlp")
                    for k in range(2):
                        nc.tensor.matmul(ps[:], w2[:, k, 128 * s:128 * (s + 1)],
                                         a1[:, k], start=(k == 0), stop=(k == 1))
                    nc.scalar.activation(a2[:, s], ps[:], AF.Tanh,
                                         bias=b2[:, s:s + 1])
                return a2

            prev = None
            for b in range(NBLOCKS + 1):
                if b < NBLOCKS:
                    xr = xr_pool.tile([SL, NSLICES, BT], bf16, tag="xr")
                    nc.sync.dma_start(xr[:], xrep_ap[b:b + 1])
                    xT = xt[:, BT * b:BT * (b + 1)]

                    h2 = mlp2(wd1, bd1, wd2, bd2, xT, "h")
                    g2 = mlp2(wo1, bo1, wo2, bo2, xT, "g")

                    # diag = (relu(d + bdo) + dm) * x  (feature-major)
                    psd = ps_mlp.tile([N, BT], f32, tag="mlp")
                    for k in range(2):
                        nc.tensor.matmul(psd[:], wdo[:, k, :], h2[:, k],
                                         start=(k == 0), stop=(k == 1))
                    dr = small_pool.tile([N, BT], f32, tag="dr")
                    nc.scalar.activation(dr[:], psd[:], AF.Relu, bias=bdo[:, 0:1])
                    dd = small_pool.tile([N, BT], f32, tag="dd")
                    nc.vector.tensor_scalar_add(dd[:], dr[:], dm[:, 0:1])
                    diag = small_pool.tile([N, BT], f32, tag="diag")
                    nc.vector.tensor_mul(out=diag[:], in0=dd[:], in1=xT)

                    off = off_pool.tile([SL, NSLICES, BT], bf16, tag="off")
                    psv = ps_acc.tile([N, BT], f32, tag="acc")

                # ---- merged slice loop: pass 1 of block b, pass 2 of b-1 ----
                for s in range(NSLICES):
                    if b < NBLOCKS:
                        pso = ps_off.tile([SL, BT], f32, tag="off")
                        for k in range(2):
                            nc.tensor.matmul(pso[:],
                                             woo[:, k, SL * s:SL * (s + 1)],
                                             g2[:, k], start=(k == 0),
                                             stop=(k == 1))
                        if s % 3 == 2:  # spread bias-adds over DVE too
                            nc.vector.tensor_scalar_add(off[:, s], pso[:],
                                                        boo[:, s:s + 1])
                        else:
                            nc.scalar.add(off[:, s], pso[:], boo[:, s:s + 1])
                        m1 = m_pool.tile([SL, BT], bf16, tag="m1")
                        nc.vector.tensor_mul(out=m1[:], in0=off[:, s],
                                             in1=xr[:, s])
                        nc.tensor.matmul(psv[:], ecol[:, N * s:N * (s + 1)],
                                         m1[:], start=(s == 0),
                                         stop=(s == NSLICES - 1))
                    if prev is not None:
                        offp, vreps, diagp, vp, pso2 = prev
                        m2 = m_pool.tile([SL, BT], bf16, tag="m2")
                        nc.vector.tensor_mul(out=m2[:], in0=offp[:, s],
                                             in1=vreps[s // CSL][:, s % CSL])
                        nc.tensor.matmul(pso2[:], erow[:, N * s:N * (s + 1)],
                                         m2[:], start=(s == 0),
                                         stop=(s == NSLICES - 1))

                # ---- epilogue of block b-1: out = pso2 + diag*v, store ----
                if prev is not None:
                    offp, vreps, diagp, vp, pso2 = prev
                    bp = b - 1
                    dvv = small_pool.tile([N, BT], f32, tag="dvv")
                    nc.vector.tensor_mul(out=dvv[:], in0=diagp[:], in1=vp[:])
                    outf = small_pool.tile([N, BT], f32, tag="outf")
                    nc.vector.tensor_add(out=outf[:], in0=pso2[:], in1=dvv[:])
                    nc.sync.dma_start(out_ap[:, BT * bp:BT * (bp + 1)], outf[:])

                # ---- v = psv + diag*x; roundtrip to DRAM; gather vrep ----
                if b < NBLOCKS:
                    dvx = small_pool.tile([N, BT], f32, tag="dvx")
                    nc.vector.tensor_mul(out=dvx[:], in0=diag[:], in1=xT)
                    v = small_pool.tile([N, BT], bf16, tag="v")
                    nc.vector.tensor_add(out=v[:], in0=psv[:], in1=dvx[:])
                    vd = vd_pool.tile([N, BT], bf16, tag="vd")
                    nc.sync.dma_start(vd[:], v[:])
                    vreps = []
                    for c in range(NCHUNK):
                        vr = vr_pool.tile([SL, CSL, BT], bf16, tag=f"vr{c}")
                        nc.gpsimd.dma_gather(
                            vr[:], vd[:], cidx[:, 32 * c:32 * (c + 1)],
                            SL * CSL, SL * CSL, BT)
                        vreps.append(vr)
                    pso2 = ps_acc.tile([N, BT], f32, tag="acc")
                    prev = (off, vreps, diag, v, pso2)

    nc.compile()
    return nc


def _get_program():
    global _compiled
    if _compiled is None:
        _compiled = _build_program()
    return _compiled


def _host_consts(inputs):
    import ml_dtypes
    f = np.float32
    bf = ml_dtypes.bfloat16
    rows, cols = np.tril_indices(N, k=-1)         # length 2016
    # padded index arrays: entries p >= 2016 are dead (all matrices zero there)
    npad = OFFP - len(rows)                        # 32

    rows_p = np.concatenate([rows, np.zeros(npad, int)])
    cols_p = np.concatenate([cols, np.zeros(npad, int)])

    ecol = np.zeros((SL, NSLICES, N), f)
    erow = np.zeros((SL, NSLICES, N), f)
    for s in range(NSLICES):
        for m in range(SL):
            p = SL * s + m
            if p < len(rows):
                ecol[m, s, cols[p]] = 1.0
                erow[m, s, rows[p]] = 1.0

    # gather index table: chunk c gathers p in [512c, 512c+512); gathered
    # vector i (= p - 512c) comes from idxs[i % 16, 32c + i // 16]
    cidx = np.zeros((128, 128), np.int16)
    for c in range(NCHUNK):
        cidx[:16, 32 * c:32 * (c + 1)] = \
            cols_p[512 * c:512 * (c + 1)].reshape(32, 16).T
    # the SWDGE gather reads a 16-partition index stripe per Q7 core;
    # replicate the table across all 8 stripes
    for c2 in range(1, 8):
        cidx[16 * c2:16 * (c2 + 1)] = cidx[:16]

    woo_pad = np.zeros((H, OFFP), f)
    woo_pad[:, :OFF] = np.asarray(inputs["Woo"], f)
    boo_pad = np.zeros(OFFP, f)
    boo_pad[:OFF] = np.asarray(inputs["boo"], f)

    def kt(w):  # [256, M] -> [128, 2, M]
        w = np.asarray(w, f)
        return np.ascontiguousarray(w.reshape(2, 128, -1).transpose(1, 0, 2))

    def bt(v):  # [256] -> [128, 2]
        return np.ascontiguousarray(np.asarray(v, f).reshape(2, 128).T)

    return rows_p, {
        "wd1": np.asarray(inputs["Wd1"], f).astype(bf),
        "wd2": kt(inputs["Wd2"]).astype(bf),
        "wdo": kt(inputs["Wdo"]).astype(bf),
        "wo1": np.asarray(inputs["Wo1"], f).astype(bf),
        "wo2": kt(inputs["Wo2"]).astype(bf),
        "woo": kt(woo_pad).astype(bf),
        "bd1": bt(inputs["bd1"]),
        "bd2": bt(inputs["bd2"]),
        "bo1": bt(inputs["bo1"]),
        "bo2": bt(inputs["bo2"]),
        "bdo": np.asarray(inputs["bdo"], f).reshape(N, 1),
        "boo": np.ascontiguousarray(boo_pad.reshape(NSLICES, SL).T),
        "dm": np.asarray(inputs["damp_min"], f).reshape(N, 1),
        "cidx": cidx,
        "ecol": np.ascontiguousarray(ecol.reshape(SL, NSLICES * N)).astype(bf),
        "erow": np.ascontiguousarray(erow.reshape(SL, NSLICES * N)).astype(bf),
    }


def kernel(trace=False, **inputs):
    import ml_dtypes
    from concourse.bass_utils import run_bass_kernel_spmd

    bf = ml_dtypes.bfloat16
    nc = _get_program()
    rows_p, consts = _host_consts(inputs)
    # feature-major bf16 x for the whole batch: [64, 32768]
    xt_all = np.ascontiguousarray(
        np.asarray(inputs["x"], np.float32).T).astype(bf)
    in_maps = []
    for i in range(NCORES):
        xtc = xt_all[:, i * BLOCAL:(i + 1) * BLOCAL]       # [64, 4096]
        # xrep[b, q, s, t] = x[sample 512b+t, feature rows_p[128s+q]]
        xg = xtc[rows_p]                                   # [2048, 4096]
        xg = xg.reshape(NSLICES, SL, NBLOCKS, BT)          # [s, q, b, t]
        xrep = np.ascontiguousarray(xg.transpose(2, 1, 0, 3))
        in_maps.append({"xt": np.ascontiguousarray(xtc), "xrep": xrep,
                        **consts})
    res = run_bass_kernel_spmd(nc, in_maps, core_ids=list(range(NCORES)),
                               trace=trace)
    out = np.concatenate(
        [np.asarray(res.results[i]["out"]).T for i in range(NCORES)], axis=0)
    if trace:
        kernel.last_results = res
    return out
